# revision 13
# baseline (speedup 1.0000x reference)
"""Trainium2 Bass kernel for nn_NCFG_21139829031662 (gnn_message_passing).

v2: replaces per-column indirect_dma_start entity gathers (10.8 ns/row,
serial on SWDGE queue 0) with class-split dma_gather + dma_scatter_add
permutes spread over all 4 SWDGE queues (4 Q7 core pairs generating
descriptors in parallel; queues 1-3 retire from the Pool engine in ~400ns).

Mechanics per gather set (heads/tails per hop, hop0 seeds):
  - entity rows are fetched as 256B elements (2 rows) from two DRAM views
    (A = rows 0.., B = rows 1..) so the wanted row is always the first
    128B; int16 gather indices address 32768-unit segments -> 16 classes
    (2 views x 8 segments). Tokens are class-sorted on host, padded to
    CAP per class with -1 (trimmed by ucode).
  - a dma_scatter_add (SBUF parity-split dst, unique slots = pure permute)
    places each token at its G-layout slot: col j<128 -> E buffer, j>=128
    -> O buffer, 64-f32 slots (first 32 useful).
  - WAW dependencies between the 16 scatters of a set are stripped (slots
    are disjoint); explicit deps keep the zero-memset ordering and a join
    on the last scatter covers all queues.
Compute is the baseline pipeline (JB=16), fed by per-batch compaction
copies from the E/O buffers.
"""

import sys
import numpy as np

sys.path.insert(0, "/opt/trn_rl_repo")

# ---------------------------------------------------------------- constants
DIM = 32
N_ENTITY = 500000
N_RELATION = 64
N_USER = 100000
N_ITEM = 200000
B = 4096
K = 64
L = 2
NCORES = 8
P = 128
CLS = 16
CAP = 2304            # padded class capacity (18*128); binomial tail safe
CADJ = CAP // 16      # idx cols per class in wrap16 layout
NSETS = 5             # h0, t0, h1, t1, seeds
SEGU = 32767          # units per segment (idx 0 = zero element)


def build_core_program(BC=512, JB=16):
    import concourse.bass as bass
    import concourse.bacc as bacc
    import concourse.mybir as mybir
    import concourse.tile as tile
    from concourse.library_config import mlp
    from concourse.tile_rust import add_dep_helper

    J = BC // 2              # 256 j-columns
    NBATCH = J // JB         # 16
    NCHUNK = J // 16         # 16
    NR = 2 * NCHUNK          # 32
    assert J % JB == 0 and JB % 16 == 0
    CPB = JB // 16           # 1
    STB = JB // 4            # 4 supertiles per batch
    f32 = mybir.dt.float32
    bf16 = mybir.dt.bfloat16
    i32 = mybir.dt.int32
    i16 = mybir.dt.int16

    nc = bacc.Bacc("TRN2", target_bir_lowering=False, debug=False,
                   num_swdge_queues=4)

    ent_seg = nc.dram_tensor("ent_seg", [CLS, 32768, 64], f32,
                             kind="ExternalInput").ap()
    rec_user = nc.dram_tensor("rec_user", [N_USER, DIM], f32, kind="ExternalInput").ap()
    item_comb = nc.dram_tensor("item_comb", [N_ITEM, DIM], f32,
                               kind="ExternalInput").ap()
    gidx_d = nc.dram_tensor("gidx", [NSETS, P, CLS * CADJ], i16,
                            kind="ExternalInput").ap()
    sidx_d = nc.dram_tensor("sidx", [NSETS, P, CLS * CADJ], i16,
                            kind="ExternalInput").ap()
    rrsq_in = nc.dram_tensor("rrsq", [L, P, J], f32, kind="ExternalInput").ap()
    onehot_in = nc.dram_tensor("onehot", [L, N_RELATION, J * P], bf16,
                               kind="ExternalInput").ap()
    fin_users = nc.dram_tensor("fin_users", [P, 4], i32, kind="ExternalInput").ap()
    fin_items = nc.dram_tensor("fin_items", [P, 4], i32, kind="ExternalInput").ap()
    wh_bd = nc.dram_tensor("wh_bd", [P, P], f32, kind="ExternalInput").ap()
    whh_bd = nc.dram_tensor("whh_bd", [P, P], f32, kind="ExternalInput").ap()
    wrtab_in = nc.dram_tensor("wrtab", [N_RELATION, DIM], bf16,
                              kind="ExternalInput").ap()
    ident_in = nc.dram_tensor("ident_in", [P, P], f32, kind="ExternalInput").ap()
    b2_in = nc.dram_tensor("b2", [P, 1], f32, kind="ExternalInput").ap()
    sels_in = nc.dram_tensor("sels", [P, NCHUNK * NR], f32, kind="ExternalInput").ap()
    par2_in = nc.dram_tensor("par2", [P, 2], f32, kind="ExternalInput").ap()
    parT_in = nc.dram_tensor("parT", [2, P], f32, kind="ExternalInput").ap()
    out_dram = nc.dram_tensor("scores", [NR, 16], f32, kind="ExternalOutput").ap()

    def raw(x):
        return getattr(x, "ins", x)

    with tile.TileContext(nc) as tc:
        with (
            tc.tile_pool(name="const", bufs=1) as cpool,
            tc.tile_pool(name="idx", bufs=2) as ipool,
            tc.tile_pool(name="cls", bufs=1) as clspool,
            tc.tile_pool(name="G", bufs=1) as Gpool,
            tc.tile_pool(name="oh", bufs=2) as ohpool,
            tc.tile_pool(name="work", bufs=2) as wpool,
            tc.tile_pool(name="small", bufs=2) as spool,
            tc.tile_pool(name="psO", bufs=1, space="PSUM") as poolO,
            tc.tile_pool(name="psT", bufs=2, space="PSUM") as poolT,
            tc.tile_pool(name="psR", bufs=1, space="PSUM") as poolR,
            tc.tile_pool(name="psS", bufs=1, space="PSUM") as poolS,
        ):
            nc.gpsimd.load_library(mlp)

            # final-gather indices first (small indirect gathers on the
            # stock SWDGE path, ~11us, before the gather storm)
            fu = ipool.tile([P, 4], i32, tag="fu")
            nc.sync.dma_start(out=fu[:], in_=fin_users[:, :])
            fi = ipool.tile([P, 4], i32, tag="fi")
            nc.sync.dma_start(out=fi[:], in_=fin_items[:, :])

            ident = cpool.tile([P, P], f32, tag="ident")
            nc.sync.dma_start(out=ident[:], in_=ident_in[:, :])
            wh_t = cpool.tile([P, P], f32, tag="wh")
            nc.sync.dma_start(out=wh_t[:], in_=wh_bd[:, :])
            whh_t = cpool.tile([P, P], f32, tag="whh")
            nc.sync.dma_start(out=whh_t[:], in_=whh_bd[:, :])
            wrtab_t = cpool.tile([N_RELATION, DIM], bf16, tag="wrtab")
            nc.sync.dma_start(out=wrtab_t[:], in_=wrtab_in[:, :])
            b2_t = cpool.tile([P, 1], f32, tag="b2")
            nc.sync.dma_start(out=b2_t[:], in_=b2_in[:, :])
            sels_t = cpool.tile([P, NCHUNK * NR], f32, tag="sels")
            nc.sync.dma_start(out=sels_t[:], in_=sels_in[:, :])
            par2_t = cpool.tile([P, 2], f32, tag="par2")
            nc.sync.dma_start(out=par2_t[:], in_=par2_in[:, :])
            parT_t = cpool.tile([2, P], f32, tag="parT")
            nc.sync.dma_start(out=parT_t[:], in_=parT_in[:, :])
            rr_full = [cpool.tile([P, J], f32, tag=f"rrf{l}", name=f"rrf{l}")
                       for l in range(L)]
            for l in range(L):
                nc.sync.dma_start(out=rr_full[l][:], in_=rrsq_in[l, :, :])

            # persistent G-layout buffers (64-f32 token slots, first 32 used)
            HE = Gpool.tile([P, J // 2, 64], f32, tag="HE")
            HO = Gpool.tile([P, J // 2, 64], f32, tag="HO")
            TE = Gpool.tile([P, J // 2, 64], f32, tag="TE")
            TO = Gpool.tile([P, J // 2, 64], f32, tag="TO")

            QORDER = [1, 2, 3, 0]
            strip_names = []

            # 3 class tiles hard-bound to queues 1..3 (queue 0 never used:
            # q0 calls block the Pool engine for their full gen time)
            ct_tiles = [clspool.tile([P, CAP // 128, 64], f32, tag=f"cls{i}",
                                     name=f"cls{i}") for i in range(3)]

            def pipe_set(si, Eb, Ob):
                git = ipool.tile([P, CLS * CADJ], i16, tag="gix")
                nc.sync.dma_start(out=git[:], in_=gidx_d[si, :, :])
                sit = ipool.tile([P, CLS * CADJ], i16, tag="six")
                nc.sync.dma_start(out=sit[:], in_=sidx_d[si, :, :])
                mE = raw(nc.vector.memset(Eb[:], 0.0))
                mO = raw(nc.vector.memset(Ob[:], 0.0))
                perq = {}
                for c in range(CLS):
                    i3 = (c + si) % 3
                    q = i3 + 1
                    ct = ct_tiles[i3]
                    bg = nc.gpsimd.dma_gather(
                        ct[:], ent_seg[c, :, :],
                        git[:, c * CADJ:(c + 1) * CADJ],
                        CAP, CAP, 64, single_packet=False, queue_num=q)
                    rg = raw(bg)
                    for nm in strip_names:
                        rg.try_remove_dependency(nm)
                    strip_names.append(rg.name)
                    bi = nc.gpsimd.dma_scatter_add(
                        Eb[:], ct[:],
                        sit[:, c * CADJ:(c + 1) * CADJ],
                        CAP, CAP, 64,
                        sbuf_tokens_per_rank=128, parity_reg=0,
                        out_ap_other=Ob[:], queue_num=q)
                    r = raw(bi)
                    for nm in strip_names:
                        r.try_remove_dependency(nm)
                    add_dep_helper(r, mE, reason="zero before scatter")
                    add_dep_helper(r, mO, reason="zero before scatter")
                    strip_names.append(r.name)
                    perq[q] = r
                return list(perq.values())

            # ---------------- final gathers + ru.ie partial (baseline)
            ru_p = spool.tile([P, 4 * DIM], f32, tag="rup")
            ie_p = spool.tile([P, 4 * DIM], f32, tag="iep")
            for cb in range(4):
                sl = slice(cb * DIM, (cb + 1) * DIM)
                nc.gpsimd.indirect_dma_start(
                    out=ru_p[:, sl], out_offset=None, in_=rec_user[:, :],
                    in_offset=bass.IndirectOffsetOnAxis(
                        ap=fu[:, cb:cb + 1], axis=0))
                nc.gpsimd.indirect_dma_start(
                    out=ie_p[:, sl], out_offset=None, in_=item_comb[:, :],
                    in_offset=bass.IndirectOffsetOnAxis(
                        ap=fi[:, cb:cb + 1], axis=0))
            ru_g = spool.tile([NR, 512], f32, tag="ru")
            ie_g = spool.tile([NR, 512], f32, tag="ie")
            for cb in range(4):
                nc.sync.dma_start(
                    out=ru_g[:, cb * 128:(cb + 1) * 128],
                    in_=ru_p[:, cb * DIM:(cb + 1) * DIM])
                nc.sync.dma_start(
                    out=ie_g[:, cb * 128:(cb + 1) * 128],
                    in_=ie_p[:, cb * DIM:(cb + 1) * DIM])
            prB = spool.tile([NR, 512], f32, tag="prB")
            nc.vector.tensor_tensor(out=prB[:], in0=ru_g[:], in1=ie_g[:],
                                    op=mybir.AluOpType.mult)
            dotB = spool.tile([NR, 16], f32, tag="dotB")
            nc.vector.tensor_reduce(
                out=dotB[:], in_=prB[:].rearrange("p (j d) -> p j d", d=DIM),
                axis=mybir.AxisListType.X, op=mybir.AluOpType.add)

            # persistent output accumulator
            o_ps = poolO.tile([NR, 512], f32, tag="o")
            first_omm = [True]

            def o_accum(rhs_ap, chunk, is_last):
                nc.tensor.matmul(
                    out=o_ps[:, :],
                    lhsT=sels_t[:, chunk * NR:(chunk + 1) * NR],
                    rhs=rhs_ap,
                    start=first_omm[0],
                    stop=is_last,
                    skip_group_check=True,
                )
                first_omm[0] = False

            def hop_compute(l, Bh_E, Bh_O, Bt_E, Bt_O, jh, jt):
                for b in range(NBATCH):
                    jlo = b * JB
                    half = b >= NBATCH // 2
                    c0 = jlo % (J // 2)
                    bh = Bh_O if half else Bh_E
                    bt = Bt_O if half else Bt_E
                    Hg = wpool.tile([P, JB * DIM], f32, tag="h")
                    bch = nc.vector.tensor_copy(
                        out=Hg[:].rearrange("p (j d) -> p j d", d=DIM),
                        in_=bh[:, c0:c0 + JB, :DIM])
                    for r_ in jh:
                        add_dep_helper(raw(bch), r_, reason="scatter join h")
                    Tg = wpool.tile([P, JB * DIM], f32, tag="t")
                    bct = nc.scalar.copy(
                        out=Tg[:].rearrange("p (j d) -> p j d", d=DIM),
                        in_=bt[:, c0:c0 + JB, :DIM])
                    for r_ in jt:
                        add_dep_helper(raw(bct), r_, reason="scatter join t")
                    oh = ohpool.tile([N_RELATION, JB * P], bf16, tag="oh")
                    nc.sync.dma_start(
                        out=oh[:], in_=onehot_in[l, :, jlo * P:(jlo + JB) * P])

                    prod = wpool.tile([P, JB * DIM], f32, tag="sc")
                    nc.vector.tensor_tensor(
                        out=prod[:], in0=Hg[:], in1=Tg[:],
                        op=mybir.AluOpType.mult)
                    dht = spool.tile([P, JB], f32, tag="dht")
                    nc.vector.tensor_reduce(
                        out=dht[:],
                        in_=prod[:].rearrange("p (j d) -> p j d", d=DIM),
                        axis=mybir.AxisListType.X, op=mybir.AluOpType.add)
                    logits = spool.tile([P, JB], f32, tag="lg")
                    nc.vector.tensor_tensor(
                        out=logits[:], in0=dht[:],
                        in1=rr_full[l][:, jlo:jlo + JB], op=mybir.AluOpType.add)
                    E = spool.tile([P, JB], f32, tag="E")
                    nc.scalar.activation(
                        out=E[:], in_=logits[:],
                        func=mybir.ActivationFunctionType.Exp)
                    den_ps = poolS.tile([2, JB], f32, tag="dn")
                    nc.tensor.matmul(out=den_ps[:], lhsT=par2_t[:], rhs=E[:],
                                     start=True, stop=True)
                    rec = spool.tile([2, JB], f32, tag="rec")
                    nc.vector.reciprocal(out=rec[:], in_=den_ps[:])
                    rb_ps = poolS.tile([P, JB], f32, tag="rb")
                    nc.tensor.matmul(out=rb_ps[:], lhsT=parT_t[:], rhs=rec[:],
                                     start=True, stop=True)
                    pi = spool.tile([P, JB], f32, tag="pi")
                    nc.vector.tensor_tensor(
                        out=pi[:], in0=E[:], in1=rb_ps[:],
                        op=mybir.AluOpType.mult)

                    HgT = wpool.tile([P, JB * DIM], f32, tag="hT")
                    TgT = wpool.tile([P, JB * DIM], f32, tag="tT")
                    for (src, dst, ei) in ((Hg, HgT, 0), (Tg, TgT, 1)):
                        for g in range(STB // 4):
                            tp = poolT.tile([P, 512], f32, tag="tp")
                            for q in range(4):
                                st = g * 4 + q
                                nc.tensor.transpose(
                                    out=tp[:, q * 128:(q + 1) * 128],
                                    in_=src[:, st * 128:(st + 1) * 128],
                                    identity=ident[:])
                            if (g + ei) % 2 == 0:
                                nc.vector.tensor_copy(
                                    out=dst[:, g * 512:(g + 1) * 512], in_=tp[:])
                            else:
                                nc.scalar.copy(
                                    out=dst[:, g * 512:(g + 1) * 512], in_=tp[:])

                    A_ps = poolR.tile([P, JB * DIM], f32, tag="rnn")
                    for st in range(STB):
                        for q in range(4):
                            jc = st * 4 + q
                            nc.tensor.matmul(
                                out=A_ps[q * DIM:(q + 1) * DIM,
                                         st * 128:(st + 1) * 128],
                                lhsT=wrtab_t[:],
                                rhs=oh[:, jc * P:(jc + 1) * P],
                                start=True, stop=False,
                                tile_position=(0, q * DIM),
                                skip_group_check=True)
                    for st in range(STB):
                        nc.tensor.matmul(
                            out=A_ps[:, st * 128:(st + 1) * 128], lhsT=wh_t[:],
                            rhs=HgT[:, st * 128:(st + 1) * 128],
                            start=False, stop=(st % 4 == 3),
                            skip_group_check=True)
                    h1 = wpool.tile([P, JB * DIM], f32, tag="hT")
                    nc.scalar.activation(
                        out=h1[:], in_=A_ps[:],
                        func=mybir.ActivationFunctionType.Tanh, bias=b2_t[:, :])

                    B_ps = poolR.tile([P, JB * DIM], f32, tag="rnn")
                    for st in range(STB):
                        for q in range(4):
                            jc = st * 4 + q
                            nc.tensor.matmul(
                                out=B_ps[q * DIM:(q + 1) * DIM,
                                         st * 128:(st + 1) * 128],
                                lhsT=wrtab_t[:],
                                rhs=oh[:, jc * P:(jc + 1) * P],
                                start=True, stop=False,
                                tile_position=(0, q * DIM),
                                skip_group_check=True)
                    for st in range(STB):
                        nc.tensor.matmul(
                            out=B_ps[:, st * 128:(st + 1) * 128], lhsT=wh_t[:],
                            rhs=TgT[:, st * 128:(st + 1) * 128],
                            start=False, stop=False,
                            skip_group_check=True)
                    for st in range(STB):
                        nc.tensor.matmul(
                            out=B_ps[:, st * 128:(st + 1) * 128], lhsT=whh_t[:],
                            rhs=h1[:, st * 128:(st + 1) * 128],
                            start=False, stop=(st % 4 == 3),
                            skip_group_check=True)
                    h2T = wpool.tile([P, JB * DIM], f32, tag="tT")
                    nc.scalar.activation(
                        out=h2T[:], in_=B_ps[:],
                        func=mybir.ActivationFunctionType.Tanh, bias=b2_t[:, :])

                    C_ps = poolR.tile([P, JB * DIM], f32, tag="rnn")
                    for st in range(STB):
                        nc.tensor.transpose(
                            out=C_ps[:, st * 128:(st + 1) * 128],
                            in_=h2T[:, st * 128:(st + 1) * 128],
                            identity=ident[:])
                    scaled = wpool.tile([P, JB * DIM], f32, tag="sc")
                    for c in range(CPB):
                        nc.vector.tensor_tensor(
                            out=scaled[:, c * 512:(c + 1) * 512].rearrange(
                                "p (j d) -> p j d", d=DIM),
                            in0=C_ps[:, c * 512:(c + 1) * 512].rearrange(
                                "p (j d) -> p j d", d=DIM),
                            in1=pi[:, c * 16:(c + 1) * 16][:, :, None].to_broadcast(
                                [P, 16, DIM]),
                            op=mybir.AluOpType.mult)
                    for c in range(CPB):
                        o_accum(scaled[:, c * 512:(c + 1) * 512],
                                b * CPB + c, False)

            # ---------------- pipeline
            j0 = pipe_set(0, HE, HO)      # heads l=0
            j1 = pipe_set(1, TE, TO)      # tails l=0
            hop_compute(0, HE, HO, TE, TO, j0, j1)
            j2 = pipe_set(2, HE, HO)      # heads l=1
            j3 = pipe_set(3, TE, TO)      # tails l=1
            hop_compute(1, HE, HO, TE, TO, j2, j3)
            j4 = pipe_set(4, HE, HO)      # seeds

            # hop0 seed-sum accumulation (last, closes the o_ps group)
            for b in range(NBATCH):
                jlo = b * JB
                half = b >= NBATCH // 2
                c0 = jlo % (J // 2)
                bh = HO if half else HE
                g0 = wpool.tile([P, JB * DIM], f32, tag="h")
                bc0 = nc.vector.tensor_copy(
                    out=g0[:].rearrange("p (j d) -> p j d", d=DIM),
                    in_=bh[:, c0:c0 + JB, :DIM])
                for r_ in j4:
                    add_dep_helper(raw(bc0), r_, reason="scatter join seeds")
                for c in range(CPB):
                    o_accum(g0[:, c * 512:(c + 1) * 512], b * CPB + c,
                            (b == NBATCH - 1) and (c == CPB - 1))

            # ---------------- final: sigmoid(o.ie + ru.ie)
            pr = spool.tile([NR, 512], f32, tag="pr")
            nc.vector.tensor_tensor(out=pr[:], in0=o_ps[:], in1=ie_g[:],
                                    op=mybir.AluOpType.mult)
            sc = spool.tile([NR, 16], f32, tag="scs")
            nc.vector.tensor_reduce(
                out=sc[:], in_=pr[:].rearrange("p (j d) -> p j d", d=DIM),
                axis=mybir.AxisListType.X, op=mybir.AluOpType.add)
            sc2 = spool.tile([NR, 16], f32, tag="sc2")
            nc.vector.tensor_tensor(out=sc2[:], in0=sc[:], in1=dotB[:],
                                    op=mybir.AluOpType.add)
            sg = spool.tile([NR, 16], f32, tag="sg")
            nc.scalar.activation(out=sg[:], in_=sc2[:],
                                 func=mybir.ActivationFunctionType.Sigmoid)
            nc.sync.dma_start(out=out_dram[:, :], in_=sg[:])

    nc.compile()
    return nc


# ---------------------------------------------------------------- host prep
def _prep_core_inputs(c, BC, users, items, hop0_items, heads, relations, tails,
                      entity_emb, relation_emb, rec_user_emb, rec_item_emb,
                      W_ih, W_hh, b_ih, b_hh, JB=16):
    import ml_dtypes
    J = BC // 2
    NCHUNK = J // 16
    NR = 2 * NCHUNK
    lo, hi = c * BC, (c + 1) * BC

    def glayout(a, dtype=np.int64):  # [BC, K] -> [128, J]
        return np.ascontiguousarray(
            a.reshape(J, 2, K).transpose(1, 2, 0).reshape(P, J)).astype(dtype)

    def flayout2(a):
        p = np.arange(P)[:, None]
        cb = np.arange(4)[None, :]
        r, q = p // 4, p % 4
        j = cb * 4 + q
        u = (r // 2) * 32 + j * 2 + (r % 2)
        return np.ascontiguousarray(np.asarray(a)[u]).astype(np.int32)

    # class-split gather/scatter index arrays
    p_g, j_g = np.meshgrid(np.arange(P), np.arange(J), indexing="ij")
    slot_g = (p_g + 128 * (2 * (j_g % (J // 2)) + (j_g // (J // 2)))).astype(np.int64)

    def class_split(iG):
        e = iG.astype(np.int64)
        v = e & 1
        u = e >> 1
        seg = u // SEGU
        cls = v * 8 + seg
        lidx = u - seg * SEGU + 1
        g = np.zeros((CLS, CAP), np.int64)     # pad gather idx 0 = zero elem
        s = np.zeros((CLS, CAP), np.int64)
        for cc in range(CLS):
            m = cls == cc
            n = int(m.sum())
            assert 0 < n <= CAP, (cc, n)
            g[cc, :n] = lidx[m]
            s[cc, :n] = slot_g[m]
            s[cc, n:] = s[cc, 0]               # pads: +0 onto an earlier real slot

        def w16(a):
            out = np.zeros((P, CLS * CADJ), np.int16)
            for cc in range(CLS):
                out[:, cc * CADJ:(cc + 1) * CADJ] = np.tile(
                    a[cc].astype(np.int16).reshape(-1, 16).T, (8, 1))
            return out
        return w16(g), w16(s)

    sets = [glayout(heads[0, lo:hi]), glayout(tails[0, lo:hi]),
            glayout(heads[1, lo:hi]), glayout(tails[1, lo:hi]),
            glayout(hop0_items[lo:hi])]
    gs = [class_split(x) for x in sets]
    gidx = np.stack([g for g, s in gs])
    sidx = np.stack([s for g, s in gs])

    rel_g = np.stack([glayout(relations[l, lo:hi]) for l in range(L)])
    rr_tab = (relation_emb.astype(np.float64) ** 2).sum(axis=1).astype(np.float32)
    rrsq = rr_tab[rel_g]

    oh = (rel_g[:, None, :, :] == np.arange(N_RELATION)[None, :, None, None])
    oh = oh.transpose(0, 1, 3, 2).reshape(L, N_RELATION, J * P)
    oh = np.ascontiguousarray(oh).astype(ml_dtypes.bfloat16)

    Wh = W_ih[:, :DIM]
    Wr = W_ih[:, DIM:]
    wrtab = (relation_emb @ Wr.T).astype(ml_dtypes.bfloat16)

    def blockdiag(w):
        m = np.zeros((P, P), np.float32)
        for j in range(4):
            m[j * 32:(j + 1) * 32, j * 32:(j + 1) * 32] = w.T
        return m

    b2 = np.tile((b_ih + b_hh).astype(np.float32), 4)[:, None]

    sels = np.zeros((P, NCHUNK, NR), np.float32)
    pvec = np.arange(P) // 64
    for m in range(NCHUNK):
        for p in range(P):
            sels[p, m, 2 * m + pvec[p]] = 1.0
    par2 = np.zeros((P, 2), np.float32)
    par2[np.arange(P), pvec] = 1.0

    if "ent_seg" not in _ENT_CACHE:
        npad = 2 * (8 * SEGU + 1) + 2
        ent_pad = np.zeros((npad, DIM), np.float32)
        ent_pad[:N_ENTITY] = entity_emb
        seg_list = []
        for v in range(2):
            flat = ent_pad[v:v + 2 * (8 * SEGU + 1)].reshape(8 * SEGU + 1, 64)
            for sgi in range(8):
                blk = np.zeros((32768, 64), np.float32)
                blk[1:1 + SEGU] = flat[sgi * SEGU:(sgi + 1) * SEGU]
                seg_list.append(blk)
        _ENT_CACHE["ent_seg"] = np.ascontiguousarray(np.stack(seg_list))
    ent_seg = _ENT_CACHE["ent_seg"]

    return {
        "ent_seg": ent_seg,
        "rec_user": np.ascontiguousarray(rec_user_emb, np.float32),
        "item_comb": np.ascontiguousarray(
            entity_emb[:N_ITEM] + rec_item_emb, np.float32),
        "gidx": gidx,
        "sidx": sidx,
        "rrsq": rrsq,
        "onehot": oh,
        "fin_users": flayout2(users[lo:hi]),
        "fin_items": flayout2(items[lo:hi]),
        "wh_bd": blockdiag(Wh),
        "whh_bd": blockdiag(W_hh),
        "wrtab": wrtab,
        "ident_in": np.eye(P, dtype=np.float32),
        "b2": b2,
        "sels": np.ascontiguousarray(sels.reshape(P, NCHUNK * NR)),
        "par2": par2,
        "parT": np.ascontiguousarray(par2.T),
    }


def _unscramble(out_c, BC):
    NCHUNK = (BC // 2) // 16
    return np.ascontiguousarray(
        out_c.reshape(NCHUNK, 2, 16).transpose(0, 2, 1).reshape(BC))


_CACHED = {}
_ENT_CACHE = {}
TRACE = False
LAST_RESULTS = None


def kernel(**inputs):
    global LAST_RESULTS
    from concourse import bass_utils

    BC = B // NCORES
    if "nc" not in _CACHED:
        _CACHED["nc"] = build_core_program(BC=BC)
    nc = _CACHED["nc"]

    args = {k: np.asarray(v) for k, v in inputs.items()}
    in_maps = [
        _prep_core_inputs(
            c, BC,
            args["users"], args["items"], args["hop0_items"], args["heads"],
            args["relations"], args["tails"],
            np.asarray(args["entity_emb"], np.float32),
            np.asarray(args["relation_emb"], np.float32),
            np.asarray(args["rec_user_emb"], np.float32),
            np.asarray(args["rec_item_emb"], np.float32),
            np.asarray(args["W_ih"], np.float32),
            np.asarray(args["W_hh"], np.float32),
            np.asarray(args["b_ih"], np.float32),
            np.asarray(args["b_hh"], np.float32),
        )
        for c in range(NCORES)
    ]
    res = bass_utils.run_bass_kernel_spmd(
        nc, in_maps, core_ids=list(range(NCORES)), trace=TRACE)
    LAST_RESULTS = res
    out = np.concatenate(
        [_unscramble(res.results[c]["scores"], BC) for c in range(NCORES)])
    return out


# revision 14
# speedup vs baseline: 1.1882x; 1.1882x over previous
"""Trainium2 Bass kernel for nn_NCFG_21139829031662 (gnn_message_passing).

v2: replaces per-column indirect_dma_start entity gathers (10.8 ns/row,
serial on SWDGE queue 0) with class-split dma_gather + dma_scatter_add
permutes spread over all 4 SWDGE queues (4 Q7 core pairs generating
descriptors in parallel; queues 1-3 retire from the Pool engine in ~400ns).

Mechanics per gather set (heads/tails per hop, hop0 seeds):
  - entity rows are fetched as 256B elements (2 rows) from two DRAM views
    (A = rows 0.., B = rows 1..) so the wanted row is always the first
    128B; int16 gather indices address 32768-unit segments -> 16 classes
    (2 views x 8 segments). Tokens are class-sorted on host, padded to
    CAP per class with -1 (trimmed by ucode).
  - a dma_scatter_add (SBUF parity-split dst, unique slots = pure permute)
    places each token at its G-layout slot: col j<128 -> E buffer, j>=128
    -> O buffer, 64-f32 slots (first 32 useful).
  - WAW dependencies between the 16 scatters of a set are stripped (slots
    are disjoint); explicit deps keep the zero-memset ordering and a join
    on the last scatter covers all queues.
Compute is the baseline pipeline (JB=16), fed by per-batch compaction
copies from the E/O buffers.
"""

import sys
import numpy as np

sys.path.insert(0, "/opt/trn_rl_repo")

# ---------------------------------------------------------------- constants
DIM = 32
N_ENTITY = 500000
N_RELATION = 64
N_USER = 100000
N_ITEM = 200000
B = 4096
K = 64
L = 2
NCORES = 8
P = 128
CLS = 16
CAP = 2304            # padded class capacity (18*128); binomial tail safe
CADJ = CAP // 16      # idx cols per class in wrap16 layout
NSETS = 5             # h0, t0, h1, t1, seeds
SEGU = 32767          # units per segment (idx 0 = zero element)


def build_core_program(BC=512, JB=16):
    import concourse.bass as bass
    import concourse.bacc as bacc
    import concourse.mybir as mybir
    import concourse.tile as tile
    from concourse.library_config import mlp
    from concourse.tile_rust import add_dep_helper

    J = BC // 2              # 256 j-columns
    NBATCH = J // JB         # 16
    NCHUNK = J // 16         # 16
    NR = 2 * NCHUNK          # 32
    assert J % JB == 0 and JB % 16 == 0
    CPB = JB // 16           # 1
    STB = JB // 4            # 4 supertiles per batch
    f32 = mybir.dt.float32
    bf16 = mybir.dt.bfloat16
    i32 = mybir.dt.int32
    i16 = mybir.dt.int16

    nc = bacc.Bacc("TRN2", target_bir_lowering=False, debug=False,
                   num_swdge_queues=4)

    ent_seg = nc.dram_tensor("ent_seg", [CLS, 32768, 64], f32,
                             kind="ExternalInput").ap()
    rec_user = nc.dram_tensor("rec_user", [N_USER, DIM], f32, kind="ExternalInput").ap()
    item_comb = nc.dram_tensor("item_comb", [N_ITEM, DIM], f32,
                               kind="ExternalInput").ap()
    gidx_d = nc.dram_tensor("gidx", [NSETS, P, CLS * CADJ], i16,
                            kind="ExternalInput").ap()
    sidx_d = nc.dram_tensor("sidx", [NSETS, P, CLS * CADJ], i16,
                            kind="ExternalInput").ap()
    rrsq_in = nc.dram_tensor("rrsq", [L, P, J], f32, kind="ExternalInput").ap()
    onehot_in = nc.dram_tensor("onehot", [L, N_RELATION, J * P], bf16,
                               kind="ExternalInput").ap()
    fin_users = nc.dram_tensor("fin_users", [P, 4], i32, kind="ExternalInput").ap()
    fin_items = nc.dram_tensor("fin_items", [P, 4], i32, kind="ExternalInput").ap()
    wh_bd = nc.dram_tensor("wh_bd", [P, P], f32, kind="ExternalInput").ap()
    whh_bd = nc.dram_tensor("whh_bd", [P, P], f32, kind="ExternalInput").ap()
    wrtab_in = nc.dram_tensor("wrtab", [N_RELATION, DIM], bf16,
                              kind="ExternalInput").ap()
    ident_in = nc.dram_tensor("ident_in", [P, P], f32, kind="ExternalInput").ap()
    b2_in = nc.dram_tensor("b2", [P, 1], f32, kind="ExternalInput").ap()
    sels_in = nc.dram_tensor("sels", [P, NCHUNK * NR], f32, kind="ExternalInput").ap()
    par2_in = nc.dram_tensor("par2", [P, 2], f32, kind="ExternalInput").ap()
    parT_in = nc.dram_tensor("parT", [2, P], f32, kind="ExternalInput").ap()
    out_dram = nc.dram_tensor("scores", [NR, 16], f32, kind="ExternalOutput").ap()

    def raw(x):
        return getattr(x, "ins", x)

    with tile.TileContext(nc) as tc:
        with (
            tc.tile_pool(name="const", bufs=1) as cpool,
            tc.tile_pool(name="idx", bufs=2) as ipool,
            tc.tile_pool(name="cls", bufs=1) as clspool,
            tc.tile_pool(name="G", bufs=1) as Gpool,
            tc.tile_pool(name="oh", bufs=1) as ohpool,
            tc.tile_pool(name="work", bufs=2) as wpool,
            tc.tile_pool(name="small", bufs=2) as spool,
            tc.tile_pool(name="psO", bufs=1, space="PSUM") as poolO,
            tc.tile_pool(name="psT", bufs=2, space="PSUM") as poolT,
            tc.tile_pool(name="psR", bufs=1, space="PSUM") as poolR,
            tc.tile_pool(name="psS", bufs=1, space="PSUM") as poolS,
        ):
            nc.gpsimd.load_library(mlp)

            # final-gather indices first (small indirect gathers on the
            # stock SWDGE path, ~11us, before the gather storm)
            fu = ipool.tile([P, 4], i32, tag="fu")
            nc.sync.dma_start(out=fu[:], in_=fin_users[:, :])
            fi = ipool.tile([P, 4], i32, tag="fi")
            nc.sync.dma_start(out=fi[:], in_=fin_items[:, :])

            ident = cpool.tile([P, P], f32, tag="ident")
            nc.sync.dma_start(out=ident[:], in_=ident_in[:, :])
            wh_t = cpool.tile([P, P], f32, tag="wh")
            nc.sync.dma_start(out=wh_t[:], in_=wh_bd[:, :])
            whh_t = cpool.tile([P, P], f32, tag="whh")
            nc.sync.dma_start(out=whh_t[:], in_=whh_bd[:, :])
            wrtab_t = cpool.tile([N_RELATION, DIM], bf16, tag="wrtab")
            nc.sync.dma_start(out=wrtab_t[:], in_=wrtab_in[:, :])
            b2_t = cpool.tile([P, 1], f32, tag="b2")
            nc.sync.dma_start(out=b2_t[:], in_=b2_in[:, :])
            sels_t = cpool.tile([P, NCHUNK * NR], f32, tag="sels")
            nc.sync.dma_start(out=sels_t[:], in_=sels_in[:, :])
            par2_t = cpool.tile([P, 2], f32, tag="par2")
            nc.sync.dma_start(out=par2_t[:], in_=par2_in[:, :])
            parT_t = cpool.tile([2, P], f32, tag="parT")
            nc.sync.dma_start(out=parT_t[:], in_=parT_in[:, :])
            rr_full = [cpool.tile([P, J], f32, tag=f"rrf{l}", name=f"rrf{l}")
                       for l in range(L)]
            for l in range(L):
                nc.sync.dma_start(out=rr_full[l][:], in_=rrsq_in[l, :, :])

            # persistent G-layout buffers (64-f32 token slots, first 32 used)
            HE = Gpool.tile([P, J // 2, 64], f32, tag="HE")
            HO = Gpool.tile([P, J // 2, 64], f32, tag="HO")
            TE = Gpool.tile([P, J // 2, 64], f32, tag="TE")
            TO = Gpool.tile([P, J // 2, 64], f32, tag="TO")

            QORDER = [1, 2, 3, 0]
            strip_names = []

            # 3 class tiles hard-bound to queues 1..3 (queue 0 never used:
            # q0 calls block the Pool engine for their full gen time)
            ct_tiles = [clspool.tile([P, CAP // 128, 64], f32, tag=f"cls{i}",
                                     name=f"cls{i}") for i in range(3)]

            def pipe_set(si, Eb, Ob):
                git = ipool.tile([P, CLS * CADJ], i16, tag="gix")
                nc.sync.dma_start(out=git[:], in_=gidx_d[si, :, :])
                sit = ipool.tile([P, CLS * CADJ], i16, tag="six")
                nc.sync.dma_start(out=sit[:], in_=sidx_d[si, :, :])
                mE = raw(nc.vector.memset(Eb[:], 0.0))
                mO = raw(nc.vector.memset(Ob[:], 0.0))
                perq = {}
                for c in range(CLS):
                    i3 = (c + si) % 3
                    q = i3 + 1
                    ct = ct_tiles[i3]
                    bg = nc.gpsimd.dma_gather(
                        ct[:], ent_seg[c, :, :],
                        git[:, c * CADJ:(c + 1) * CADJ],
                        CAP, CAP, 64, single_packet=False, queue_num=q)
                    rg = raw(bg)
                    for nm in strip_names:
                        rg.try_remove_dependency(nm)
                    strip_names.append(rg.name)
                    bi = nc.gpsimd.dma_scatter_add(
                        Eb[:], ct[:],
                        sit[:, c * CADJ:(c + 1) * CADJ],
                        CAP, CAP, 64,
                        sbuf_tokens_per_rank=128, parity_reg=0,
                        out_ap_other=Ob[:], queue_num=q)
                    r = raw(bi)
                    for nm in strip_names:
                        r.try_remove_dependency(nm)
                    add_dep_helper(r, mE, reason="zero before scatter")
                    add_dep_helper(r, mO, reason="zero before scatter")
                    strip_names.append(r.name)
                    perq[q] = r
                return list(perq.values())

            # ---------------- final gathers + ru.ie partial (baseline)
            ru_p = spool.tile([P, 4 * DIM], f32, tag="rup")
            ie_p = spool.tile([P, 4 * DIM], f32, tag="iep")
            for cb in range(4):
                sl = slice(cb * DIM, (cb + 1) * DIM)
                nc.gpsimd.indirect_dma_start(
                    out=ru_p[:, sl], out_offset=None, in_=rec_user[:, :],
                    in_offset=bass.IndirectOffsetOnAxis(
                        ap=fu[:, cb:cb + 1], axis=0))
                nc.gpsimd.indirect_dma_start(
                    out=ie_p[:, sl], out_offset=None, in_=item_comb[:, :],
                    in_offset=bass.IndirectOffsetOnAxis(
                        ap=fi[:, cb:cb + 1], axis=0))
            ru_g = spool.tile([NR, 512], f32, tag="ru")
            ie_g = spool.tile([NR, 512], f32, tag="ie")
            for cb in range(4):
                nc.sync.dma_start(
                    out=ru_g[:, cb * 128:(cb + 1) * 128],
                    in_=ru_p[:, cb * DIM:(cb + 1) * DIM])
                nc.sync.dma_start(
                    out=ie_g[:, cb * 128:(cb + 1) * 128],
                    in_=ie_p[:, cb * DIM:(cb + 1) * DIM])
            prB = spool.tile([NR, 512], f32, tag="prB")
            nc.vector.tensor_tensor(out=prB[:], in0=ru_g[:], in1=ie_g[:],
                                    op=mybir.AluOpType.mult)
            dotB = spool.tile([NR, 16], f32, tag="dotB")
            nc.vector.tensor_reduce(
                out=dotB[:], in_=prB[:].rearrange("p (j d) -> p j d", d=DIM),
                axis=mybir.AxisListType.X, op=mybir.AluOpType.add)

            # persistent output accumulator
            o_ps = poolO.tile([NR, 512], f32, tag="o")
            first_omm = [True]

            def o_accum(rhs_ap, chunk, is_last):
                nc.tensor.matmul(
                    out=o_ps[:, :],
                    lhsT=sels_t[:, chunk * NR:(chunk + 1) * NR],
                    rhs=rhs_ap,
                    start=first_omm[0],
                    stop=is_last,
                    skip_group_check=True,
                )
                first_omm[0] = False

            def hop_compute(l, Bh_E, Bh_O, Bt_E, Bt_O, jh, jt):
                for b in range(NBATCH):
                    jlo = b * JB
                    half = b >= NBATCH // 2
                    c0 = jlo % (J // 2)
                    bh = Bh_O if half else Bh_E
                    bt = Bt_O if half else Bt_E
                    Hg = wpool.tile([P, JB * DIM], f32, tag="h")
                    bch = nc.vector.tensor_copy(
                        out=Hg[:].rearrange("p (j d) -> p j d", d=DIM),
                        in_=bh[:, c0:c0 + JB, :DIM])
                    for r_ in jh:
                        add_dep_helper(raw(bch), r_, reason="scatter join h")
                    Tg = wpool.tile([P, JB * DIM], f32, tag="t")
                    bct = nc.scalar.copy(
                        out=Tg[:].rearrange("p (j d) -> p j d", d=DIM),
                        in_=bt[:, c0:c0 + JB, :DIM])
                    for r_ in jt:
                        add_dep_helper(raw(bct), r_, reason="scatter join t")
                    oh = ohpool.tile([N_RELATION, JB * P], bf16, tag="oh")
                    nc.sync.dma_start(
                        out=oh[:], in_=onehot_in[l, :, jlo * P:(jlo + JB) * P])

                    prod = wpool.tile([P, JB * DIM], f32, tag="sc")
                    nc.vector.tensor_tensor(
                        out=prod[:], in0=Hg[:], in1=Tg[:],
                        op=mybir.AluOpType.mult)
                    dht = spool.tile([P, JB], f32, tag="dht")
                    nc.vector.tensor_reduce(
                        out=dht[:],
                        in_=prod[:].rearrange("p (j d) -> p j d", d=DIM),
                        axis=mybir.AxisListType.X, op=mybir.AluOpType.add)
                    logits = spool.tile([P, JB], f32, tag="lg")
                    nc.vector.tensor_tensor(
                        out=logits[:], in0=dht[:],
                        in1=rr_full[l][:, jlo:jlo + JB], op=mybir.AluOpType.add)
                    E = spool.tile([P, JB], f32, tag="E")
                    nc.scalar.activation(
                        out=E[:], in_=logits[:],
                        func=mybir.ActivationFunctionType.Exp)
                    den_ps = poolS.tile([2, JB], f32, tag="dn")
                    nc.tensor.matmul(out=den_ps[:], lhsT=par2_t[:], rhs=E[:],
                                     start=True, stop=True)
                    rec = spool.tile([2, JB], f32, tag="rec")
                    nc.vector.reciprocal(out=rec[:], in_=den_ps[:])
                    rb_ps = poolS.tile([P, JB], f32, tag="rb")
                    nc.tensor.matmul(out=rb_ps[:], lhsT=parT_t[:], rhs=rec[:],
                                     start=True, stop=True)
                    pi = spool.tile([P, JB], f32, tag="pi")
                    nc.vector.tensor_tensor(
                        out=pi[:], in0=E[:], in1=rb_ps[:],
                        op=mybir.AluOpType.mult)

                    HgT = wpool.tile([P, JB * DIM], f32, tag="hT")
                    TgT = wpool.tile([P, JB * DIM], f32, tag="tT")
                    for (src, dst, ei) in ((Hg, HgT, 0), (Tg, TgT, 1)):
                        for g in range(STB // 4):
                            tp = poolT.tile([P, 512], f32, tag="tp")
                            for q in range(4):
                                st = g * 4 + q
                                nc.tensor.transpose(
                                    out=tp[:, q * 128:(q + 1) * 128],
                                    in_=src[:, st * 128:(st + 1) * 128],
                                    identity=ident[:])
                            if (g + ei) % 2 == 0:
                                nc.vector.tensor_copy(
                                    out=dst[:, g * 512:(g + 1) * 512], in_=tp[:])
                            else:
                                nc.scalar.copy(
                                    out=dst[:, g * 512:(g + 1) * 512], in_=tp[:])

                    A_ps = poolR.tile([P, JB * DIM], f32, tag="rnn")
                    for st in range(STB):
                        for q in range(4):
                            jc = st * 4 + q
                            nc.tensor.matmul(
                                out=A_ps[q * DIM:(q + 1) * DIM,
                                         st * 128:(st + 1) * 128],
                                lhsT=wrtab_t[:],
                                rhs=oh[:, jc * P:(jc + 1) * P],
                                start=True, stop=False,
                                tile_position=(0, q * DIM),
                                skip_group_check=True)
                    for st in range(STB):
                        nc.tensor.matmul(
                            out=A_ps[:, st * 128:(st + 1) * 128], lhsT=wh_t[:],
                            rhs=HgT[:, st * 128:(st + 1) * 128],
                            start=False, stop=(st % 4 == 3),
                            skip_group_check=True)
                    h1 = wpool.tile([P, JB * DIM], f32, tag="h1")
                    nc.scalar.activation(
                        out=h1[:], in_=A_ps[:],
                        func=mybir.ActivationFunctionType.Tanh, bias=b2_t[:, :])

                    B_ps = poolR.tile([P, JB * DIM], f32, tag="rnn")
                    for st in range(STB):
                        for q in range(4):
                            jc = st * 4 + q
                            nc.tensor.matmul(
                                out=B_ps[q * DIM:(q + 1) * DIM,
                                         st * 128:(st + 1) * 128],
                                lhsT=wrtab_t[:],
                                rhs=oh[:, jc * P:(jc + 1) * P],
                                start=True, stop=False,
                                tile_position=(0, q * DIM),
                                skip_group_check=True)
                    for st in range(STB):
                        nc.tensor.matmul(
                            out=B_ps[:, st * 128:(st + 1) * 128], lhsT=wh_t[:],
                            rhs=TgT[:, st * 128:(st + 1) * 128],
                            start=False, stop=False,
                            skip_group_check=True)
                    for st in range(STB):
                        nc.tensor.matmul(
                            out=B_ps[:, st * 128:(st + 1) * 128], lhsT=whh_t[:],
                            rhs=h1[:, st * 128:(st + 1) * 128],
                            start=False, stop=(st % 4 == 3),
                            skip_group_check=True)
                    h2T = wpool.tile([P, JB * DIM], f32, tag="h2T")
                    nc.scalar.activation(
                        out=h2T[:], in_=B_ps[:],
                        func=mybir.ActivationFunctionType.Tanh, bias=b2_t[:, :])

                    C_ps = poolR.tile([P, JB * DIM], f32, tag="rnn")
                    for st in range(STB):
                        nc.tensor.transpose(
                            out=C_ps[:, st * 128:(st + 1) * 128],
                            in_=h2T[:, st * 128:(st + 1) * 128],
                            identity=ident[:])
                    scaled = wpool.tile([P, JB * DIM], f32, tag="sc")
                    for c in range(CPB):
                        nc.vector.tensor_tensor(
                            out=scaled[:, c * 512:(c + 1) * 512].rearrange(
                                "p (j d) -> p j d", d=DIM),
                            in0=C_ps[:, c * 512:(c + 1) * 512].rearrange(
                                "p (j d) -> p j d", d=DIM),
                            in1=pi[:, c * 16:(c + 1) * 16][:, :, None].to_broadcast(
                                [P, 16, DIM]),
                            op=mybir.AluOpType.mult)
                    for c in range(CPB):
                        o_accum(scaled[:, c * 512:(c + 1) * 512],
                                b * CPB + c, False)

            # ---------------- pipeline
            j0 = pipe_set(0, HE, HO)      # heads l=0
            j1 = pipe_set(1, TE, TO)      # tails l=0
            hop_compute(0, HE, HO, TE, TO, j0, j1)
            j2 = pipe_set(2, HE, HO)      # heads l=1
            j3 = pipe_set(3, TE, TO)      # tails l=1
            hop_compute(1, HE, HO, TE, TO, j2, j3)
            j4 = pipe_set(4, HE, HO)      # seeds

            # hop0 seed-sum accumulation (last, closes the o_ps group)
            for b in range(NBATCH):
                jlo = b * JB
                half = b >= NBATCH // 2
                c0 = jlo % (J // 2)
                bh = HO if half else HE
                g0 = wpool.tile([P, JB * DIM], f32, tag="h")
                bc0 = nc.vector.tensor_copy(
                    out=g0[:].rearrange("p (j d) -> p j d", d=DIM),
                    in_=bh[:, c0:c0 + JB, :DIM])
                for r_ in j4:
                    add_dep_helper(raw(bc0), r_, reason="scatter join seeds")
                for c in range(CPB):
                    o_accum(g0[:, c * 512:(c + 1) * 512], b * CPB + c,
                            (b == NBATCH - 1) and (c == CPB - 1))

            # ---------------- final: sigmoid(o.ie + ru.ie)
            pr = spool.tile([NR, 512], f32, tag="pr")
            nc.vector.tensor_tensor(out=pr[:], in0=o_ps[:], in1=ie_g[:],
                                    op=mybir.AluOpType.mult)
            sc = spool.tile([NR, 16], f32, tag="scs")
            nc.vector.tensor_reduce(
                out=sc[:], in_=pr[:].rearrange("p (j d) -> p j d", d=DIM),
                axis=mybir.AxisListType.X, op=mybir.AluOpType.add)
            sc2 = spool.tile([NR, 16], f32, tag="sc2")
            nc.vector.tensor_tensor(out=sc2[:], in0=sc[:], in1=dotB[:],
                                    op=mybir.AluOpType.add)
            sg = spool.tile([NR, 16], f32, tag="sg")
            nc.scalar.activation(out=sg[:], in_=sc2[:],
                                 func=mybir.ActivationFunctionType.Sigmoid)
            nc.sync.dma_start(out=out_dram[:, :], in_=sg[:])

    nc.compile()
    return nc


# ---------------------------------------------------------------- host prep
def _prep_core_inputs(c, BC, users, items, hop0_items, heads, relations, tails,
                      entity_emb, relation_emb, rec_user_emb, rec_item_emb,
                      W_ih, W_hh, b_ih, b_hh, JB=16):
    import ml_dtypes
    J = BC // 2
    NCHUNK = J // 16
    NR = 2 * NCHUNK
    lo, hi = c * BC, (c + 1) * BC

    def glayout(a, dtype=np.int64):  # [BC, K] -> [128, J]
        return np.ascontiguousarray(
            a.reshape(J, 2, K).transpose(1, 2, 0).reshape(P, J)).astype(dtype)

    def flayout2(a):
        p = np.arange(P)[:, None]
        cb = np.arange(4)[None, :]
        r, q = p // 4, p % 4
        j = cb * 4 + q
        u = (r // 2) * 32 + j * 2 + (r % 2)
        return np.ascontiguousarray(np.asarray(a)[u]).astype(np.int32)

    # class-split gather/scatter index arrays
    p_g, j_g = np.meshgrid(np.arange(P), np.arange(J), indexing="ij")
    slot_g = (p_g + 128 * (2 * (j_g % (J // 2)) + (j_g // (J // 2)))).astype(np.int64)

    def class_split(iG):
        e = iG.astype(np.int64)
        v = e & 1
        u = e >> 1
        seg = u // SEGU
        cls = v * 8 + seg
        lidx = u - seg * SEGU + 1
        g = np.zeros((CLS, CAP), np.int64)     # pad gather idx 0 = zero elem
        s = np.zeros((CLS, CAP), np.int64)
        for cc in range(CLS):
            m = cls == cc
            n = int(m.sum())
            assert 0 < n <= CAP, (cc, n)
            g[cc, :n] = lidx[m]
            s[cc, :n] = slot_g[m]
            s[cc, n:] = s[cc, 0]               # pads: +0 onto an earlier real slot

        def w16(a):
            out = np.zeros((P, CLS * CADJ), np.int16)
            for cc in range(CLS):
                out[:, cc * CADJ:(cc + 1) * CADJ] = np.tile(
                    a[cc].astype(np.int16).reshape(-1, 16).T, (8, 1))
            return out
        return w16(g), w16(s)

    sets = [glayout(heads[0, lo:hi]), glayout(tails[0, lo:hi]),
            glayout(heads[1, lo:hi]), glayout(tails[1, lo:hi]),
            glayout(hop0_items[lo:hi])]
    gs = [class_split(x) for x in sets]
    gidx = np.stack([g for g, s in gs])
    sidx = np.stack([s for g, s in gs])

    rel_g = np.stack([glayout(relations[l, lo:hi]) for l in range(L)])
    rr_tab = (relation_emb.astype(np.float64) ** 2).sum(axis=1).astype(np.float32)
    rrsq = rr_tab[rel_g]

    oh = (rel_g[:, None, :, :] == np.arange(N_RELATION)[None, :, None, None])
    oh = oh.transpose(0, 1, 3, 2).reshape(L, N_RELATION, J * P)
    oh = np.ascontiguousarray(oh).astype(ml_dtypes.bfloat16)

    Wh = W_ih[:, :DIM]
    Wr = W_ih[:, DIM:]
    wrtab = (relation_emb @ Wr.T).astype(ml_dtypes.bfloat16)

    def blockdiag(w):
        m = np.zeros((P, P), np.float32)
        for j in range(4):
            m[j * 32:(j + 1) * 32, j * 32:(j + 1) * 32] = w.T
        return m

    b2 = np.tile((b_ih + b_hh).astype(np.float32), 4)[:, None]

    sels = np.zeros((P, NCHUNK, NR), np.float32)
    pvec = np.arange(P) // 64
    for m in range(NCHUNK):
        for p in range(P):
            sels[p, m, 2 * m + pvec[p]] = 1.0
    par2 = np.zeros((P, 2), np.float32)
    par2[np.arange(P), pvec] = 1.0

    if "ent_seg" not in _ENT_CACHE:
        npad = 2 * (8 * SEGU + 1) + 2
        ent_pad = np.zeros((npad, DIM), np.float32)
        ent_pad[:N_ENTITY] = entity_emb
        seg_list = []
        for v in range(2):
            flat = ent_pad[v:v + 2 * (8 * SEGU + 1)].reshape(8 * SEGU + 1, 64)
            for sgi in range(8):
                blk = np.zeros((32768, 64), np.float32)
                blk[1:1 + SEGU] = flat[sgi * SEGU:(sgi + 1) * SEGU]
                seg_list.append(blk)
        _ENT_CACHE["ent_seg"] = np.ascontiguousarray(np.stack(seg_list))
    ent_seg = _ENT_CACHE["ent_seg"]

    return {
        "ent_seg": ent_seg,
        "rec_user": np.ascontiguousarray(rec_user_emb, np.float32),
        "item_comb": np.ascontiguousarray(
            entity_emb[:N_ITEM] + rec_item_emb, np.float32),
        "gidx": gidx,
        "sidx": sidx,
        "rrsq": rrsq,
        "onehot": oh,
        "fin_users": flayout2(users[lo:hi]),
        "fin_items": flayout2(items[lo:hi]),
        "wh_bd": blockdiag(Wh),
        "whh_bd": blockdiag(W_hh),
        "wrtab": wrtab,
        "ident_in": np.eye(P, dtype=np.float32),
        "b2": b2,
        "sels": np.ascontiguousarray(sels.reshape(P, NCHUNK * NR)),
        "par2": par2,
        "parT": np.ascontiguousarray(par2.T),
    }


def _unscramble(out_c, BC):
    NCHUNK = (BC // 2) // 16
    return np.ascontiguousarray(
        out_c.reshape(NCHUNK, 2, 16).transpose(0, 2, 1).reshape(BC))


_CACHED = {}
_ENT_CACHE = {}
TRACE = False
LAST_RESULTS = None


def kernel(**inputs):
    global LAST_RESULTS
    from concourse import bass_utils

    BC = B // NCORES
    if "nc" not in _CACHED:
        _CACHED["nc"] = build_core_program(BC=BC)
    nc = _CACHED["nc"]

    args = {k: np.asarray(v) for k, v in inputs.items()}
    in_maps = [
        _prep_core_inputs(
            c, BC,
            args["users"], args["items"], args["hop0_items"], args["heads"],
            args["relations"], args["tails"],
            np.asarray(args["entity_emb"], np.float32),
            np.asarray(args["relation_emb"], np.float32),
            np.asarray(args["rec_user_emb"], np.float32),
            np.asarray(args["rec_item_emb"], np.float32),
            np.asarray(args["W_ih"], np.float32),
            np.asarray(args["W_hh"], np.float32),
            np.asarray(args["b_ih"], np.float32),
            np.asarray(args["b_hh"], np.float32),
        )
        for c in range(NCORES)
    ]
    res = bass_utils.run_bass_kernel_spmd(
        nc, in_maps, core_ids=list(range(NCORES)), trace=TRACE)
    LAST_RESULTS = res
    out = np.concatenate(
        [_unscramble(res.results[c]["scores"], BC) for c in range(NCORES)])
    return out
